# revision 49
# baseline (speedup 1.0000x reference)
"""MultiHeadAttention TRN2 Bass kernel (B=2, S=2048, D=1024, H=16, d=64).

Sharding: 8 cores = 2 (batch) x 4 (head groups of 4 heads), no collectives.
Each core computes, for its batch b and head slice hs (256 dims):
    K^T = (Wk[hs,:] @ x_k^T + bk)    [256, 2048]   (dh on partitions)
    Q^T likewise; V = x_v @ Wv[hs,:].T + bv        [2048, 256]  (s on partitions)
    per head pair (2m, 2m+1): S^T = K_h @ Q_h^T
    P^T = exp(S^T / 8)   (scores ~ N(0,1), exp is safe without max-sub)
    [O^T ; denom] = [V_h | 1]^T @ P^T   (ones column folds the softmax
                                         denominator into the PV matmul)
    O^T = O^T * (1/denom)
    y_partial = O^T.T @ Wo[:, hs].T     [2048, 1024]
Host: y[b] = sum of 4 head-group partials + bo.

Schedule: window pipeline paced by the Scalar exp floor (~135us) and the
serial PE matmul stream (~185us).  Window p emits scores of pair p while
pair p-1's PV drains one k behind (pv(p-1,k-1) follows scores(p,k), so the
window-boundary PSUM-acc eviction hides behind the first score matmul).
Softmax norm: denominators evicted to SBUF at window end, one
reciprocal_approx_fast (DVE custom op, ~5x faster than InstReciprocal),
a single K=2 PE matmul replicates both heads' recip rows across the 128
partitions, and one Pool-engine multiply applies it early next window.
V bias is folded into the PSUM eviction (Pool partition_broadcast of bv,
ones columns memset once) instead of a PE matmul per s-tile.  The tail
splits the last pair into two 256-column halves so half A's norm chain +
yproj overlap half B's PV stream; final y DMAs go out in 128-col chunks.
"""

import numpy as np
import ml_dtypes

import concourse.bass as bass
import concourse.tile as tile
import concourse.mybir as mybir
from concourse import bacc
from concourse.bass_utils import run_bass_kernel_spmd

D_MODEL = 1024
NUM_HEADS = 16
HEAD_DIM = 64
B, S = 2, 2048
N_CORES = 8
HG = 4                  # head-groups
HEADS_PER_CORE = NUM_HEADS // HG        # 4
DH = HEADS_PER_CORE * HEAD_DIM          # 256 output dims per core
KT = D_MODEL // 128                     # 8 contraction tiles
ST = S // 128                           # 16 sequence tiles
SB = S // 512                           # 4 sequence blocks of 512

F32 = mybir.dt.float32
F32R = mybir.dt.float32r
BF16 = mybir.dt.bfloat16
AF = mybir.ActivationFunctionType
BF16_NP = ml_dtypes.bfloat16

_cached_nc = None


def build_nc():
    nc = bacc.Bacc("TRN2", target_bir_lowering=False, debug=False)

    xq_t = nc.declare_dram_parameter("xq_t", [128, KT * S], BF16, isOutput=False)
    xk_t = nc.declare_dram_parameter("xk_t", [128, KT * S], BF16, isOutput=False)
    xv_t = nc.declare_dram_parameter("xv_t", [128, KT * S], BF16, isOutput=False)
    wq_t = nc.declare_dram_parameter("wq_t", [128, KT * DH], BF16, isOutput=False)
    wk_t = nc.declare_dram_parameter("wk_t", [128, KT * DH], BF16, isOutput=False)
    wv_t = nc.declare_dram_parameter("wv_t", [128, KT * DH], BF16, isOutput=False)
    wo_t = nc.declare_dram_parameter("wo_t", [128, 2 * D_MODEL], BF16, isOutput=False)
    bqk = nc.declare_dram_parameter("bqk", [128, 4], F32, isOutput=False)
    bv = nc.declare_dram_parameter("bv", [1, DH], BF16, isOutput=False)
    y = nc.declare_dram_parameter("y", [S, D_MODEL], BF16, isOutput=True)

    with tile.TileContext(nc) as tc:
        _emit(nc, tc, xq_t, xk_t, xv_t, wq_t, wk_t, wv_t, wo_t, bqk, bv, y)
    nc.compile()
    return nc


def _emit(nc, tc, xq_t, xk_t, xv_t, wq_t, wk_t, wv_t, wo_t, bqk, bv, y):
    from contextlib import ExitStack

    ctx = ExitStack()
    with ctx:
        # ---- persistent tiles -------------------------------------------
        persist = ctx.enter_context(tc.tile_pool(name="persist", bufs=1))
        qt = [persist.tile([128, S], BF16, tag=f"qt{m}", name=f"qt{m}")
              for m in range(2)]
        kt_sb = [persist.tile([128, S], BF16, tag=f"kt{m}", name=f"kt{m}")
                 for m in range(2)]
        v_sb = [persist.tile([128, HEADS_PER_CORE * 65], BF16, tag=f"v{i}",
                             name=f"v{i}") for i in range(ST)]
        ot = [persist.tile([128, S], BF16, tag=f"ot{m}", name=f"ot{m}")
              for m in range(2)]
        wo_flat = persist.tile([128, 2 * D_MODEL], BF16, tag="wof", name="wof")
        wo_r = [wo_flat[:, m * D_MODEL:(m + 1) * D_MODEL] for m in range(2)]
        ones2 = persist.tile([33, 128], F32, tag="ones2")
        ones2_r = persist.tile([33, 128], F32R, tag="ones2r")
        den2 = persist.tile([33, 512], F32, tag="den2")
        bqk_c = persist.tile([128, 4], F32, tag="bqk")  # bq|bk per-partition
        bq_c, bk_c = bqk_c[:, 0:2], bqk_c[:, 2:4]
        bv_r = persist.tile([1, DH], BF16, tag="bvr")
        bv_bc = persist.tile([128, DH], BF16, tag="bvbc")
        w_flat = {n: persist.tile([128, KT * DH], BF16,
                                  tag=f"w{n}", name=f"w{n}")
                  for n in ("k", "q", "v")}
        w3 = {n: w_flat[n].rearrange("p (k d) -> p k d", d=DH)
              for n in ("k", "q", "v")}
        x_flat = {n: persist.tile([128, KT * S], BF16,
                                  tag=f"x{n}", name=f"x{n}")
                  for n in ("k", "q", "v")}
        x3 = {n: x_flat[n].rearrange("p (k s) -> p k s", s=S)
              for n in ("k", "q", "v")}

        # ---- DMA (priority order == consumption order) ------------------
        # The lead is SP dispatch-rate bound (~650ns per dma_start), so the
        # first projections' exact needs go first: m=0 weight halves
        # (strided slice), then the x column blocks, spread across several
        # engines' DGE queues to overlap dispatch.
        def dma_cols(dst3, dram, c0, c1, eng=None, engs=None):
            dram3 = dram.rearrange("p (k s) -> p k s", s=S)
            for kp in range(KT // 2):
                e = engs[kp % len(engs)] if engs else (eng or nc.sync)
                e.dma_start(
                    dst3[:, 2 * kp:2 * kp + 2, c0:c1],
                    dram3[:, 2 * kp:2 * kp + 2, c0:c1])

        def dma_w_half(name, dram, m, eng=None):
            dram3 = dram.rearrange("p (k d) -> p k d", d=DH)
            (eng or nc.sync).dma_start(
                w3[name][:, :, m * 128:(m + 1) * 128],
                dram3[:, :, m * 128:(m + 1) * 128])

        def dma_w4(dst_tile, dram, n=4):
            w = dst_tile.shape[1] // n
            for j in range(n):
                nc.sync.dma_start(dst_tile[:, j * w:(j + 1) * w],
                                  dram[:, j * w:(j + 1) * w])

        # critical set for window 0, round-robined over the 3 DGE queues
        dma_w_half("k", wk_t, 0)                     # sync queue
        dma_cols(x3["k"], xk_t, 0, 512,
                 engs=[nc.scalar, nc.sync, nc.scalar, nc.gpsimd])
        dma_w_half("q", wq_t, 0, nc.gpsimd)          # pool queue
        dma_cols(x3["q"], xq_t, 0, 512,
                 engs=[nc.scalar, nc.sync, nc.scalar, nc.sync])
        dma_cols(x3["k"], xk_t, 512, 1024,
                 engs=[nc.gpsimd, nc.scalar, nc.sync, nc.scalar])
        nc.gpsimd.dma_start(bqk_c[:], bqk[:, :])
        dma_w_half("k", wk_t, 1, nc.gpsimd)
        dma_cols(x3["k"], xk_t, 1024, 1536, nc.scalar)
        dma_w_half("q", wq_t, 1)
        dma_cols(x3["q"], xq_t, 512, 1024, nc.gpsimd)
        dma_cols(x3["k"], xk_t, 1536, S, nc.scalar)
        dma_cols(x3["q"], xq_t, 1024, S)
        dma_w_half("v", wv_t, 0)
        dma_w_half("v", wv_t, 1)
        nc.sync.dma_start(bv_r[:], bv[:])
        dma_cols(x3["v"], xv_t, 0, S)
        dma_w4(wo_flat, wo_t)

        # ---- pipelined-body pools ---------------------------------------
        ps_s = ctx.enter_context(
            tc.tile_pool(name="pss", bufs=2, space="PSUM"))      # 4 banks
        ps_acc = ctx.enter_context(
            tc.tile_pool(name="psacc", bufs=1, space="PSUM"))    # 2 banks
        ps_w = ctx.enter_context(
            tc.tile_pool(name="psw", bufs=2, space="PSUM"))      # 2 banks
        pt_pool = ctx.enter_context(tc.tile_pool(name="pt", bufs=19))
        sm_pool = ctx.enter_context(tc.tile_pool(name="small", bufs=1))
        sm2_pool = ctx.enter_context(tc.tile_pool(name="small2", bufs=2))
        y_pool = ctx.enter_context(tc.tile_pool(name="ysb", bufs=2))

        # constants: ones2_r replicate pattern (partition 0 -> out
        # partitions 0-63, partition 32 -> 64-127; the rest zero),
        # den2 init so unused rows are finite, broadcast bv, v_sb ones
        nc.vector.memset(ones2[:], 0.0)
        nc.vector.memset(ones2[0:1, 0:64], 1.0)
        nc.vector.memset(ones2[32:33, 64:128], 1.0)
        nc.vector.tensor_copy(ones2_r[:], ones2[:])
        nc.vector.memset(den2[:], 1.0)
        nc.gpsimd.partition_broadcast(bv_bc[:], bv_r[:])
        for i in range(ST):
            vv = v_sb[i].rearrange("p (h c) -> p h c", c=65)
            nc.gpsimd.memset(vv[:, :, 64], 1.0)

        # ---- building blocks --------------------------------------------
        def proj_qk_m(name, dst, bias_c, nb, m):
            """Project one (512-col, m-half) block of Q^T or K^T (bf16)."""
            ps = ps_w.tile([128, 512], F32, tag="pw", name="pw")
            for k in range(KT):
                nc.tensor.matmul(
                    ps[:],
                    w3[name][:, k, m * 128:(m + 1) * 128],
                    x3[name][:, k, nb * 512:(nb + 1) * 512],
                    start=(k == 0), stop=(k == KT - 1),
                )
            nc.vector.tensor_scalar_add(
                dst[m][:, nb * 512:(nb + 1) * 512], ps[:],
                bias_c[:, m:m + 1])

        def v_chunk(i):
            """Project V for s-tile i into v_sb[i]; bias folded into the
            DVE eviction (bv broadcast tile), ones column pre-set."""
            ps = ps_w.tile([128, 512], F32, tag="pw", name="pw")
            for k in range(KT):
                nc.tensor.matmul(
                    ps[:, 0:256],
                    x3["v"][:, k, i * 128:(i + 1) * 128],
                    w3["v"][:, k, :],
                    start=(k == 0), stop=(k == KT - 1),
                )
            src = ps[:, 0:256].rearrange("p (h c) -> p h c", c=64)
            vv = v_sb[i].rearrange("p (h c) -> p h c", c=65)
            bvv = bv_bc.rearrange("p (h c) -> p h c", c=64)
            nc.vector.tensor_add(vv[:, :, 0:64], src, bvv)

        def scores(qb, m, k):
            """Score pair (heads 2m,2m+1), sk-tile k, sq-block qb."""
            ss = ps_s.tile([128, 1024], F32, tag="ss", name="ss")
            for p2 in range(2):
                po = 64 * p2
                nc.tensor.matmul(
                    ss[:, p2 * 512:(p2 + 1) * 512],
                    kt_sb[m][po:po + 64, k * 128:(k + 1) * 128],
                    qt[m][po:po + 64, qb * 512:(qb + 1) * 512],
                    start=True, stop=True,
                )
            pt = pt_pool.tile([128, 1024], BF16, tag="pt", name="pt")
            nc.scalar.activation(
                pt[:], ss[:], AF.Exp, scale=1.0 / float(np.sqrt(HEAD_DIM)))
            return pt

        def pv(m, k, pt, accs, c0=0, c1=512, start=None, stop=None):
            """PV for both heads of pair-half m over pt columns [c0:c1)."""
            for p2 in range(2):
                h = 2 * m + p2
                nc.tensor.matmul(
                    accs[p2][:, 0:c1 - c0],
                    v_sb[k][:, h * 65:(h + 1) * 65],
                    pt[:, p2 * 512 + c0:p2 * 512 + c1],
                    start=(k == 0) if start is None else start,
                    stop=(k == ST - 1) if stop is None else stop,
                )

        def norm_stage1(accs, w=512, c0=0, act_evict=False):
            """Evict O rows + denominators to SBUF (frees the PSUM accs)
            and compute the batched approx reciprocal on DVE.  With
            act_evict the O-row copies ride the (post-exp idle) Scalar
            engine instead of the congested DVE queue."""
            osb = sm2_pool.tile([128, 512], BF16, tag="osb", name="osb")
            recip2 = sm_pool.tile([33, 512], F32, tag="recipf", name="recipf")
            recip2_r = sm2_pool.tile([33, 512], F32R, tag="recip2",
                                     name="recip2")
            for p2 in range(2):
                if act_evict:
                    nc.scalar.copy(osb[64 * p2:64 * p2 + 64, 0:w],
                                   accs[p2][0:64, c0:c0 + w])
                else:
                    nc.vector.tensor_copy(osb[64 * p2:64 * p2 + 64, 0:w],
                                          accs[p2][0:64, c0:c0 + w])
                nc.vector.tensor_copy(den2[32 * p2:32 * p2 + 1, 0:w],
                                      accs[p2][64:65, c0:c0 + w])
            with nc.allow_low_precision(reason="softmax denom"):
                nc.vector.reciprocal_approx_fast(recip2[:, 0:w], den2[:, 0:w])
                nc.vector.tensor_copy(recip2_r[:, 0:w], recip2[:, 0:w])
            return (osb, recip2_r)

        def norm_apply(qb, m, st, oc=0, w=512):
            """ot[m][:, qb block cols oc:oc+w] = O^T * recip: one K=2 PE
            matmul replicates both heads' recip rows; one Pool multiply."""
            osb, recip2 = st
            rep = ps_w.tile([128, 512], F32, tag="pw", name="pw")
            nc.tensor.matmul(
                rep[:, 0:w], ones2_r[:],
                recip2[:, 0:w],
                start=True, stop=True,
            )
            rep_sb = sm_pool.tile([128, 512], BF16, tag="repsb", name="repsb")
            nc.vector.tensor_copy(rep_sb[:, 0:w], rep[:, 0:w])
            nc.gpsimd.tensor_mul(
                ot[m][:, qb * 512 + oc:qb * 512 + oc + w],
                osb[:, 0:w], rep_sb[:, 0:w])

        def yproj_i(i, ysb_holder, eng=None, act_evict=False):
            """Output projection for s-tile i; DMA per half as it lands."""
            if ysb_holder[0] is None:
                ysb_holder[0] = y_pool.tile([128, D_MODEL], BF16, tag="ysb",
                                            name="ysb")
            ysb = ysb_holder[0]
            for nb2 in range(2):
                ps = ps_w.tile([128, 512], F32, tag="pw", name="pw")
                for m in range(2):
                    nc.tensor.matmul(
                        ps[:],
                        ot[m][:, i * 128:(i + 1) * 128],
                        wo_r[m][:, nb2 * 512:(nb2 + 1) * 512],
                        start=(m == 0), stop=(m == 1),
                    )
                if act_evict:
                    nc.scalar.copy(ysb[:, nb2 * 512:(nb2 + 1) * 512], ps[:])
                else:
                    nc.vector.tensor_copy(
                        ysb[:, nb2 * 512:(nb2 + 1) * 512], ps[:])
                (eng or nc.sync).dma_start(
                    y[i * 128:(i + 1) * 128, nb2 * 512:(nb2 + 1) * 512],
                    ysb[:, nb2 * 512:(nb2 + 1) * 512])
            ysb_holder[0] = None

        # =============== emission schedule ===============================
        # Window p: scores(p, k) leads pv(p-1, k-1) by one k so the
        # boundary acc eviction hides behind the first score matmul.
        pairs = [(qb, m) for qb in range(SB) for m in range(2)]
        yh = [None]

        def alloc_accs():
            return [ps_acc.tile([65, 512], F32, tag=f"acc{pp}",
                                name=f"acc{pp}") for pp in range(2)]

        proj_slots = {
            (0, 1): ("k", 0, 1),   # kt m0 nb1, read from scores(0,4)
            (0, 3): ("k", 1, 0),   # window-1 operands
            (0, 5): ("q", 1, 0),
            (0, 6): ("k", 0, 2),   # read from scores(0,8)
            (0, 7): ("k", 1, 1),
            (0, 9): ("k", 0, 3),   # read from scores(0,12)
            (0, 10): ("k", 1, 2),
            (0, 12): ("k", 1, 3),
            (0, 11): ("q", 0, 1),  # window-2 operand
            (0, 13): ("q", 1, 1),
            (2, 6): ("q", 0, 2), (2, 11): ("q", 1, 2),
            (4, 6): ("q", 0, 3), (4, 11): ("q", 1, 3),
        }
        yproj_slots = {
            (3, 6): 0, (3, 9): 1, (3, 12): 2, (4, 2): 3,     # yproj(0)
            (5, 6): 4, (5, 9): 5, (5, 12): 6, (6, 2): 7,     # yproj(1)
            (7, 6): 8, (7, 9): 9, (7, 12): 10, (7, 14): 11,  # yproj(2)
        }

        # lead-in: K and Q m=0 of block 0 only (the m=1 halves are
        # window-0 slots), so the first matmul starts on minimal DMA.
        proj_qk_m("k", kt_sb, bk_c, 0, 0)
        proj_qk_m("q", qt, bq_c, 0, 0)

        pts_prev = None
        accs_run = None
        apply_q = []            # FIFO of (qb, m, stage1 state)
        for p in range(len(pairs)):
            qb, m = pairs[p]
            if p >= 1:
                accs_run = alloc_accs()
            pts_cur = []
            for k in range(ST):
                pts_cur.append(scores(qb, m, k))
                if k == 5 and apply_q:
                    norm_apply(*apply_q.pop(0))
                if p == 1:
                    v_chunk(k)
                if p >= 1 and k >= 3:
                    pv(pairs[p - 1][1], k - 3, pts_prev[k - 3], accs_run)
                if (p, k) in proj_slots:
                    nm, pm, pnb = proj_slots[(p, k)]
                    proj_qk_m(nm, kt_sb if nm == "k" else qt,
                              bk_c if nm == "k" else bq_c, pnb, pm)
                if (p, k) in yproj_slots:
                    yproj_i(yproj_slots[(p, k)], yh)
            if p >= 1:
                for kk in range(ST - 3, ST):
                    pv(pairs[p - 1][1], kk, pts_prev[kk], accs_run)
                st = norm_stage1(accs_run,
                                 act_evict=(p == len(pairs) - 1))
                apply_q.append((pairs[p - 1][0], pairs[p - 1][1], st))
            pts_prev = pts_cur

        # ---- tail: PV + norm of the last pair (3,1), split into two
        # 256-col halves so half A's norm chain + yproj(12,13) overlap
        # half B's PV stream.
        qb_l, m_l = pairs[-1]
        accsA = alloc_accs()
        for k in range(ST):
            pv(m_l, k, pts_prev[k], accsA, c0=0, c1=256)
            if k == 5 and apply_q:
                norm_apply(*apply_q.pop(0))
        stA = norm_stage1(accsA, w=256, act_evict=True)

        ssb = ps_s.tile([128, 1024], F32, tag="ss", name="ss")
        accsB = [ssb[0:65, 0:256], ssb[0:65, 512:768]]
        for k in range(ST):
            pv(m_l, k, pts_prev[k], accsB, c0=256, c1=512)
            if k == 3:
                norm_apply(qb_l, m_l, stA, oc=0, w=256)
            if k == 6:
                yproj_i(4 * qb_l + 0, yh, act_evict=True)
            if k == 11:
                yproj_i(4 * qb_l + 1, yh, act_evict=True)
        # endgame: two 128-col pieces of half B so yproj(14) starts
        # while piece 2's norm chain is still in flight
        stB1 = norm_stage1(accsB, w=128, c0=0, act_evict=True)
        norm_apply(qb_l, m_l, stB1, oc=256, w=128)
        stB2 = norm_stage1(accsB, w=128, c0=128, act_evict=True)
        yproj_i(4 * qb_l + 2, yh, act_evict=True)
        norm_apply(qb_l, m_l, stB2, oc=384, w=128)
        yproj_i(4 * qb_l + 3, yh, eng=nc.gpsimd, act_evict=True)


def _get_nc():
    global _cached_nc
    if _cached_nc is None:
        _cached_nc = build_nc()
    return _cached_nc


def _make_in_maps(query, key, value, Wq, bq, Wk, bk, Wv, bv, Wo):
    """Shard + transpose + bf16-cast on host: core c = (b, hg), b = c // HG."""
    query = np.asarray(query, dtype=np.float32)
    key = np.asarray(key, dtype=np.float32)
    value = np.asarray(value, dtype=np.float32)
    Wq, Wk, Wv, Wo = (np.asarray(w, dtype=np.float32) for w in (Wq, Wk, Wv, Wo))
    bq, bk, bv = (np.asarray(b_, dtype=np.float32) for b_ in (bq, bk, bv))
    in_maps = []

    def tile_x(xt, dt):      # [1024, 2048] -> [128, 8*2048] k-tiled
        return np.ascontiguousarray(
            xt.reshape(KT, 128, S).transpose(1, 0, 2).reshape(128, KT * S)
        ).astype(dt)

    xq_t = [tile_x(query[b].T, BF16_NP) for b in range(B)]
    xk_t = [tile_x(key[b].T, BF16_NP) for b in range(B)]
    xv_t = [tile_x(value[b].T, BF16_NP) for b in range(B)]

    def tile_w(WT, dt=BF16_NP):  # [1024, 256] -> [128, 8*256] k-tiled
        return np.ascontiguousarray(
            WT.reshape(KT, 128, DH).transpose(1, 0, 2).reshape(128, KT * DH)
        ).astype(dt)

    for c in range(N_CORES):
        b, hg = divmod(c, HG)
        hs = slice(hg * DH, (hg + 1) * DH)
        wo_tiled = np.ascontiguousarray(
            Wo[:, hs].T.reshape(2, 128, D_MODEL).transpose(1, 0, 2)
            .reshape(128, 2 * D_MODEL)).astype(BF16_NP)
        bqk_pack = np.concatenate(
            [bq[hs].reshape(2, 128).T, bk[hs].reshape(2, 128).T],
            axis=1)          # [128, 4] = bq cols | bk cols
        in_maps.append({
            "xq_t": xq_t[b],
            "xk_t": xk_t[b],
            "xv_t": xv_t[b],
            "wq_t": tile_w(Wq[hs, :].T),
            "wk_t": tile_w(Wk[hs, :].T),
            "wv_t": tile_w(Wv[hs, :].T),
            "wo_t": wo_tiled,
            "bqk": np.ascontiguousarray(bqk_pack),
            "bv": np.ascontiguousarray(bv[hs]).reshape(1, DH).astype(BF16_NP),
        })
    return in_maps


def run(inputs, trace=False, **spmd_kwargs):
    nc = _get_nc()
    in_maps = _make_in_maps(
        inputs["query"], inputs["key"], inputs["value"],
        inputs["Wq"], inputs["bq"], inputs["Wk"], inputs["bk"],
        inputs["Wv"], inputs["bv"], inputs["Wo"])
    res = run_bass_kernel_spmd(
        nc, in_maps, list(range(N_CORES)), trace=trace, **spmd_kwargs)
    bo = np.asarray(inputs["bo"], dtype=np.float32)
    out = np.empty((B, S, D_MODEL), dtype=np.float32)
    for b in range(B):
        acc = np.zeros((S, D_MODEL), dtype=np.float32)
        for hg in range(HG):
            acc += np.asarray(res.results[b * HG + hg]["y"], dtype=np.float32)
        out[b] = acc + bo
    return out, res


def kernel(**inputs) -> np.ndarray:
    out, _ = run(inputs, trace=False)
    return out


# revision 50
# speedup vs baseline: 1.0102x; 1.0102x over previous
"""MultiHeadAttention TRN2 Bass kernel (B=2, S=2048, D=1024, H=16, d=64).

Sharding: 8 cores = 2 (batch) x 4 (head groups of 4 heads), no collectives.
Each core computes, for its batch b and head slice hs (256 dims):
    K^T = (Wk[hs,:] @ x_k^T + bk)    [256, 2048]   (dh on partitions)
    Q^T likewise; V = x_v @ Wv[hs,:].T + bv        [2048, 256]  (s on partitions)
    per head pair (2m, 2m+1): S^T = K_h @ Q_h^T
    P^T = exp(S^T / 8)   (scores ~ N(0,1), exp is safe without max-sub)
    [O^T ; denom] = [V_h | 1]^T @ P^T   (ones column folds the softmax
                                         denominator into the PV matmul)
    O^T = O^T * (1/denom)
    y_partial = O^T.T @ Wo[:, hs].T     [2048, 1024]
Host: y[b] = sum of 4 head-group partials + bo.

Schedule: window pipeline paced by the Scalar exp floor (~135us) and the
serial PE matmul stream (~177us).  Window p emits scores of pair p while
pair p-1's PV drains three k behind (pv(p-1,k-3) follows scores(p,k), so
the window-boundary PSUM-acc eviction hides behind the first score
matmuls).  Softmax norm: denominators evicted to SBUF at window end, one
reciprocal_approx_fast (DVE custom op, ~5x faster than InstReciprocal),
a single K=2 PE matmul replicates both heads' recip rows across the 128
partitions, and one Pool-engine multiply applies it early next window.
V bias is folded into the PSUM eviction (Pool partition_broadcast of bv,
ones columns memset once) instead of a PE matmul per s-tile.  Head DMA:
window-0's exact operands (m=0 weight halves as strided slices + the
first x column blocks) go first, round-robined over the SP/ACT/Pool DGE
queues (each sustains ~95GB/s with queue depth 2).  The tail splits the
last pair into two 256-col halves, then half B's norm into two 128-col
pieces, pipelining each norm chain + yproj against the next PV stream;
tail evictions ride the post-exp idle Scalar engine (scalar.copy), and
y is written back in bf16 to halve the final DMA drain.
"""

import numpy as np
import ml_dtypes

import concourse.bass as bass
import concourse.tile as tile
import concourse.mybir as mybir
from concourse import bacc
from concourse.bass_utils import run_bass_kernel_spmd

D_MODEL = 1024
NUM_HEADS = 16
HEAD_DIM = 64
B, S = 2, 2048
N_CORES = 8
HG = 4                  # head-groups
HEADS_PER_CORE = NUM_HEADS // HG        # 4
DH = HEADS_PER_CORE * HEAD_DIM          # 256 output dims per core
KT = D_MODEL // 128                     # 8 contraction tiles
ST = S // 128                           # 16 sequence tiles
SB = S // 512                           # 4 sequence blocks of 512

F32 = mybir.dt.float32
F32R = mybir.dt.float32r
BF16 = mybir.dt.bfloat16
AF = mybir.ActivationFunctionType
BF16_NP = ml_dtypes.bfloat16

_cached_nc = None


def build_nc():
    nc = bacc.Bacc("TRN2", target_bir_lowering=False, debug=False)

    xq_t = nc.declare_dram_parameter("xq_t", [128, KT * S], BF16, isOutput=False)
    xk_t = nc.declare_dram_parameter("xk_t", [128, KT * S], BF16, isOutput=False)
    xv_t = nc.declare_dram_parameter("xv_t", [128, KT * S], BF16, isOutput=False)
    wq_t = nc.declare_dram_parameter("wq_t", [128, KT * DH], BF16, isOutput=False)
    wk_t = nc.declare_dram_parameter("wk_t", [128, KT * DH], BF16, isOutput=False)
    wv_t = nc.declare_dram_parameter("wv_t", [128, KT * DH], BF16, isOutput=False)
    wo_t = nc.declare_dram_parameter("wo_t", [128, 2 * D_MODEL], BF16, isOutput=False)
    bqk = nc.declare_dram_parameter("bqk", [128, 4], F32, isOutput=False)
    bv = nc.declare_dram_parameter("bv", [1, DH], BF16, isOutput=False)
    y = nc.declare_dram_parameter("y", [S, D_MODEL], BF16, isOutput=True)

    with tile.TileContext(nc) as tc:
        _emit(nc, tc, xq_t, xk_t, xv_t, wq_t, wk_t, wv_t, wo_t, bqk, bv, y)
    nc.compile()
    return nc


def _emit(nc, tc, xq_t, xk_t, xv_t, wq_t, wk_t, wv_t, wo_t, bqk, bv, y):
    from contextlib import ExitStack

    ctx = ExitStack()
    with ctx:
        # ---- persistent tiles -------------------------------------------
        persist = ctx.enter_context(tc.tile_pool(name="persist", bufs=1))
        qt = [persist.tile([128, S], BF16, tag=f"qt{m}", name=f"qt{m}")
              for m in range(2)]
        kt_sb = [persist.tile([128, S], BF16, tag=f"kt{m}", name=f"kt{m}")
                 for m in range(2)]
        v_sb = [persist.tile([128, HEADS_PER_CORE * 65], BF16, tag=f"v{i}",
                             name=f"v{i}") for i in range(ST)]
        ot = [persist.tile([128, S], BF16, tag=f"ot{m}", name=f"ot{m}")
              for m in range(2)]
        wo_flat = persist.tile([128, 2 * D_MODEL], BF16, tag="wof", name="wof")
        wo_r = [wo_flat[:, m * D_MODEL:(m + 1) * D_MODEL] for m in range(2)]
        ones2 = persist.tile([33, 128], F32, tag="ones2")
        ones2_r = persist.tile([33, 128], F32R, tag="ones2r")
        den2 = persist.tile([33, 512], F32, tag="den2")
        bqk_c = persist.tile([128, 4], F32, tag="bqk")  # bq|bk per-partition
        bq_c, bk_c = bqk_c[:, 0:2], bqk_c[:, 2:4]
        bv_r = persist.tile([1, DH], BF16, tag="bvr")
        bv_bc = persist.tile([128, DH], BF16, tag="bvbc")
        w_flat = {n: persist.tile([128, KT * DH], BF16,
                                  tag=f"w{n}", name=f"w{n}")
                  for n in ("k", "q", "v")}
        w3 = {n: w_flat[n].rearrange("p (k d) -> p k d", d=DH)
              for n in ("k", "q", "v")}
        x_flat = {n: persist.tile([128, KT * S], BF16,
                                  tag=f"x{n}", name=f"x{n}")
                  for n in ("k", "q", "v")}
        x3 = {n: x_flat[n].rearrange("p (k s) -> p k s", s=S)
              for n in ("k", "q", "v")}

        # ---- DMA (priority order == consumption order) ------------------
        # The lead is SP dispatch-rate bound (~650ns per dma_start), so the
        # first projections' exact needs go first: m=0 weight halves
        # (strided slice), then the x column blocks, spread across several
        # engines' DGE queues to overlap dispatch.
        def dma_cols(dst3, dram, c0, c1, eng=None, engs=None):
            dram3 = dram.rearrange("p (k s) -> p k s", s=S)
            for kp in range(KT // 2):
                e = engs[kp % len(engs)] if engs else (eng or nc.sync)
                e.dma_start(
                    dst3[:, 2 * kp:2 * kp + 2, c0:c1],
                    dram3[:, 2 * kp:2 * kp + 2, c0:c1])

        def dma_w_half(name, dram, m, eng=None):
            dram3 = dram.rearrange("p (k d) -> p k d", d=DH)
            (eng or nc.sync).dma_start(
                w3[name][:, :, m * 128:(m + 1) * 128],
                dram3[:, :, m * 128:(m + 1) * 128])

        def dma_w4(dst_tile, dram, n=4):
            w = dst_tile.shape[1] // n
            for j in range(n):
                nc.sync.dma_start(dst_tile[:, j * w:(j + 1) * w],
                                  dram[:, j * w:(j + 1) * w])

        # critical set for window 0, round-robined over the 3 DGE queues
        dma_w_half("k", wk_t, 0)                     # sync queue
        dma_cols(x3["k"], xk_t, 0, 512,
                 engs=[nc.scalar, nc.sync, nc.scalar, nc.gpsimd])
        dma_w_half("q", wq_t, 0, nc.gpsimd)          # pool queue
        dma_cols(x3["q"], xq_t, 0, 512,
                 engs=[nc.scalar, nc.sync, nc.scalar, nc.sync])
        dma_cols(x3["k"], xk_t, 512, 1024,
                 engs=[nc.gpsimd, nc.scalar, nc.sync, nc.scalar])
        nc.gpsimd.dma_start(bqk_c[:], bqk[:, :])
        dma_w_half("k", wk_t, 1, nc.gpsimd)
        dma_cols(x3["k"], xk_t, 1024, 1536, nc.scalar)
        dma_w_half("q", wq_t, 1)
        dma_cols(x3["q"], xq_t, 512, 1024, nc.gpsimd)
        dma_cols(x3["k"], xk_t, 1536, S, nc.scalar)
        dma_cols(x3["q"], xq_t, 1024, S)
        dma_w_half("v", wv_t, 0)
        dma_w_half("v", wv_t, 1)
        nc.sync.dma_start(bv_r[:], bv[:])
        dma_cols(x3["v"], xv_t, 0, S)
        dma_w4(wo_flat, wo_t)

        # ---- pipelined-body pools ---------------------------------------
        ps_s = ctx.enter_context(
            tc.tile_pool(name="pss", bufs=2, space="PSUM"))      # 4 banks
        ps_acc = ctx.enter_context(
            tc.tile_pool(name="psacc", bufs=1, space="PSUM"))    # 2 banks
        ps_w = ctx.enter_context(
            tc.tile_pool(name="psw", bufs=2, space="PSUM"))      # 2 banks
        pt_pool = ctx.enter_context(tc.tile_pool(name="pt", bufs=19))
        sm_pool = ctx.enter_context(tc.tile_pool(name="small", bufs=1))
        sm2_pool = ctx.enter_context(tc.tile_pool(name="small2", bufs=2))
        y_pool = ctx.enter_context(tc.tile_pool(name="ysb", bufs=2))

        # constants: ones2_r replicate pattern (partition 0 -> out
        # partitions 0-63, partition 32 -> 64-127; the rest zero),
        # den2 init so unused rows are finite, broadcast bv, v_sb ones
        nc.vector.memset(ones2[:], 0.0)
        nc.vector.memset(ones2[0:1, 0:64], 1.0)
        nc.vector.memset(ones2[32:33, 64:128], 1.0)
        nc.vector.tensor_copy(ones2_r[:], ones2[:])
        nc.vector.memset(den2[:], 1.0)
        nc.gpsimd.partition_broadcast(bv_bc[:], bv_r[:])
        for i in range(ST):
            vv = v_sb[i].rearrange("p (h c) -> p h c", c=65)
            nc.gpsimd.memset(vv[:, :, 64], 1.0)

        # ---- building blocks --------------------------------------------
        def proj_qk_m(name, dst, bias_c, nb, m):
            """Project one (512-col, m-half) block of Q^T or K^T (bf16)."""
            ps = ps_w.tile([128, 512], F32, tag="pw", name="pw")
            for k in range(KT):
                nc.tensor.matmul(
                    ps[:],
                    w3[name][:, k, m * 128:(m + 1) * 128],
                    x3[name][:, k, nb * 512:(nb + 1) * 512],
                    start=(k == 0), stop=(k == KT - 1),
                )
            nc.vector.tensor_scalar_add(
                dst[m][:, nb * 512:(nb + 1) * 512], ps[:],
                bias_c[:, m:m + 1])

        def v_chunk(i):
            """Project V for s-tile i into v_sb[i]; bias folded into the
            DVE eviction (bv broadcast tile), ones column pre-set."""
            ps = ps_w.tile([128, 512], F32, tag="pw", name="pw")
            for k in range(KT):
                nc.tensor.matmul(
                    ps[:, 0:256],
                    x3["v"][:, k, i * 128:(i + 1) * 128],
                    w3["v"][:, k, :],
                    start=(k == 0), stop=(k == KT - 1),
                )
            src = ps[:, 0:256].rearrange("p (h c) -> p h c", c=64)
            vv = v_sb[i].rearrange("p (h c) -> p h c", c=65)
            bvv = bv_bc.rearrange("p (h c) -> p h c", c=64)
            nc.vector.tensor_add(vv[:, :, 0:64], src, bvv)

        def scores(qb, m, k):
            """Score pair (heads 2m,2m+1), sk-tile k, sq-block qb."""
            ss = ps_s.tile([128, 1024], F32, tag="ss", name="ss")
            for p2 in range(2):
                po = 64 * p2
                nc.tensor.matmul(
                    ss[:, p2 * 512:(p2 + 1) * 512],
                    kt_sb[m][po:po + 64, k * 128:(k + 1) * 128],
                    qt[m][po:po + 64, qb * 512:(qb + 1) * 512],
                    start=True, stop=True,
                )
            pt = pt_pool.tile([128, 1024], BF16, tag="pt", name="pt")
            nc.scalar.activation(
                pt[:], ss[:], AF.Exp, scale=1.0 / float(np.sqrt(HEAD_DIM)))
            return pt

        def pv(m, k, pt, accs, c0=0, c1=512, start=None, stop=None):
            """PV for both heads of pair-half m over pt columns [c0:c1)."""
            for p2 in range(2):
                h = 2 * m + p2
                nc.tensor.matmul(
                    accs[p2][:, 0:c1 - c0],
                    v_sb[k][:, h * 65:(h + 1) * 65],
                    pt[:, p2 * 512 + c0:p2 * 512 + c1],
                    start=(k == 0) if start is None else start,
                    stop=(k == ST - 1) if stop is None else stop,
                )

        def norm_stage1(accs, w=512, c0=0, act_evict=False):
            """Evict O rows + denominators to SBUF (frees the PSUM accs)
            and compute the batched approx reciprocal on DVE.  With
            act_evict the O-row copies ride the (post-exp idle) Scalar
            engine instead of the congested DVE queue."""
            osb = sm2_pool.tile([128, 512], BF16, tag="osb", name="osb")
            recip2 = sm_pool.tile([33, 512], F32, tag="recipf", name="recipf")
            recip2_r = sm2_pool.tile([33, 512], F32R, tag="recip2",
                                     name="recip2")
            for p2 in range(2):
                if act_evict:
                    nc.scalar.copy(osb[64 * p2:64 * p2 + 64, 0:w],
                                   accs[p2][0:64, c0:c0 + w])
                else:
                    nc.vector.tensor_copy(osb[64 * p2:64 * p2 + 64, 0:w],
                                          accs[p2][0:64, c0:c0 + w])
                nc.vector.tensor_copy(den2[32 * p2:32 * p2 + 1, 0:w],
                                      accs[p2][64:65, c0:c0 + w])
            with nc.allow_low_precision(reason="softmax denom"):
                nc.vector.reciprocal_approx_fast(recip2[:, 0:w], den2[:, 0:w])
                nc.vector.tensor_copy(recip2_r[:, 0:w], recip2[:, 0:w])
            return (osb, recip2_r)

        def norm_apply(qb, m, st, oc=0, w=512):
            """ot[m][:, qb block cols oc:oc+w] = O^T * recip: one K=2 PE
            matmul replicates both heads' recip rows; one Pool multiply."""
            osb, recip2 = st
            rep = ps_w.tile([128, 512], F32, tag="pw", name="pw")
            nc.tensor.matmul(
                rep[:, 0:w], ones2_r[:],
                recip2[:, 0:w],
                start=True, stop=True,
            )
            rep_sb = sm_pool.tile([128, 512], BF16, tag="repsb", name="repsb")
            nc.vector.tensor_copy(rep_sb[:, 0:w], rep[:, 0:w])
            nc.gpsimd.tensor_mul(
                ot[m][:, qb * 512 + oc:qb * 512 + oc + w],
                osb[:, 0:w], rep_sb[:, 0:w])

        def yproj_i(i, ysb_holder, eng=None, act_evict=False):
            """Output projection for s-tile i; DMA per half as it lands."""
            if ysb_holder[0] is None:
                ysb_holder[0] = y_pool.tile([128, D_MODEL], BF16, tag="ysb",
                                            name="ysb")
            ysb = ysb_holder[0]
            for nb2 in range(2):
                ps = ps_w.tile([128, 512], F32, tag="pw", name="pw")
                for m in range(2):
                    nc.tensor.matmul(
                        ps[:],
                        ot[m][:, i * 128:(i + 1) * 128],
                        wo_r[m][:, nb2 * 512:(nb2 + 1) * 512],
                        start=(m == 0), stop=(m == 1),
                    )
                if act_evict:
                    nc.scalar.copy(ysb[:, nb2 * 512:(nb2 + 1) * 512], ps[:])
                else:
                    nc.vector.tensor_copy(
                        ysb[:, nb2 * 512:(nb2 + 1) * 512], ps[:])
                (eng or nc.sync).dma_start(
                    y[i * 128:(i + 1) * 128, nb2 * 512:(nb2 + 1) * 512],
                    ysb[:, nb2 * 512:(nb2 + 1) * 512])
            ysb_holder[0] = None

        # =============== emission schedule ===============================
        # Window p: scores(p, k) leads pv(p-1, k-1) by one k so the
        # boundary acc eviction hides behind the first score matmul.
        pairs = [(qb, m) for qb in range(SB) for m in range(2)]
        yh = [None]

        def alloc_accs():
            return [ps_acc.tile([65, 512], F32, tag=f"acc{pp}",
                                name=f"acc{pp}") for pp in range(2)]

        proj_slots = {
            (0, 1): ("k", 0, 1),   # kt m0 nb1, read from scores(0,4)
            (0, 3): ("k", 1, 0),   # window-1 operands
            (0, 5): ("q", 1, 0),
            (0, 6): ("k", 0, 2),   # read from scores(0,8)
            (0, 7): ("k", 1, 1),
            (0, 9): ("k", 0, 3),   # read from scores(0,12)
            (0, 10): ("k", 1, 2),
            (0, 12): ("k", 1, 3),
            (0, 11): ("q", 0, 1),  # window-2 operand
            (0, 13): ("q", 1, 1),
            (2, 6): ("q", 0, 2), (2, 11): ("q", 1, 2),
            (4, 6): ("q", 0, 3), (4, 11): ("q", 1, 3),
        }
        yproj_slots = {
            (3, 6): 0, (3, 9): 1, (3, 12): 2, (4, 2): 3,     # yproj(0)
            (5, 6): 4, (5, 9): 5, (5, 12): 6, (6, 2): 7,     # yproj(1)
            (7, 6): 8, (7, 9): 9, (7, 12): 10, (7, 14): 11,  # yproj(2)
        }

        # lead-in: K and Q m=0 of block 0 only (the m=1 halves are
        # window-0 slots), so the first matmul starts on minimal DMA.
        proj_qk_m("k", kt_sb, bk_c, 0, 0)
        proj_qk_m("q", qt, bq_c, 0, 0)

        pts_prev = None
        accs_run = None
        apply_q = []            # FIFO of (qb, m, stage1 state)
        for p in range(len(pairs)):
            qb, m = pairs[p]
            if p >= 1:
                accs_run = alloc_accs()
            pts_cur = []
            for k in range(ST):
                pts_cur.append(scores(qb, m, k))
                if k == 5 and apply_q:
                    norm_apply(*apply_q.pop(0))
                if p == 1:
                    v_chunk(k)
                if p >= 1 and k >= 3:
                    pv(pairs[p - 1][1], k - 3, pts_prev[k - 3], accs_run)
                if (p, k) in proj_slots:
                    nm, pm, pnb = proj_slots[(p, k)]
                    proj_qk_m(nm, kt_sb if nm == "k" else qt,
                              bk_c if nm == "k" else bq_c, pnb, pm)
                if (p, k) in yproj_slots:
                    yproj_i(yproj_slots[(p, k)], yh)
            if p >= 1:
                for kk in range(ST - 3, ST):
                    pv(pairs[p - 1][1], kk, pts_prev[kk], accs_run)
                st = norm_stage1(accs_run,
                                 act_evict=(p == len(pairs) - 1))
                apply_q.append((pairs[p - 1][0], pairs[p - 1][1], st))
            pts_prev = pts_cur

        # ---- tail: PV + norm of the last pair (3,1), split into two
        # 256-col halves so half A's norm chain + yproj(12,13) overlap
        # half B's PV stream.
        qb_l, m_l = pairs[-1]
        accsA = alloc_accs()
        for k in range(ST):
            pv(m_l, k, pts_prev[k], accsA, c0=0, c1=256)
            if k == 5 and apply_q:
                norm_apply(*apply_q.pop(0))
        stA = norm_stage1(accsA, w=256, act_evict=True)

        ssb = ps_s.tile([128, 1024], F32, tag="ss", name="ss")
        accsB = [ssb[0:65, 0:256], ssb[0:65, 512:768]]
        for k in range(ST):
            pv(m_l, k, pts_prev[k], accsB, c0=256, c1=512)
            if k == 3:
                norm_apply(qb_l, m_l, stA, oc=0, w=256)
            if k == 6:
                yproj_i(4 * qb_l + 0, yh, act_evict=True)
            if k == 11:
                yproj_i(4 * qb_l + 1, yh, act_evict=True)
        # endgame: two 128-col pieces of half B so yproj(14) starts
        # while piece 2's norm chain is still in flight
        stB1 = norm_stage1(accsB, w=128, c0=0, act_evict=True)
        norm_apply(qb_l, m_l, stB1, oc=256, w=128)
        stB2 = norm_stage1(accsB, w=128, c0=128, act_evict=True)
        yproj_i(4 * qb_l + 2, yh, act_evict=True)
        norm_apply(qb_l, m_l, stB2, oc=384, w=128)
        yproj_i(4 * qb_l + 3, yh, eng=nc.gpsimd, act_evict=True)


def _get_nc():
    global _cached_nc
    if _cached_nc is None:
        _cached_nc = build_nc()
    return _cached_nc


def _make_in_maps(query, key, value, Wq, bq, Wk, bk, Wv, bv, Wo):
    """Shard + transpose + bf16-cast on host: core c = (b, hg), b = c // HG."""
    query = np.asarray(query, dtype=np.float32)
    key = np.asarray(key, dtype=np.float32)
    value = np.asarray(value, dtype=np.float32)
    Wq, Wk, Wv, Wo = (np.asarray(w, dtype=np.float32) for w in (Wq, Wk, Wv, Wo))
    bq, bk, bv = (np.asarray(b_, dtype=np.float32) for b_ in (bq, bk, bv))
    in_maps = []

    def tile_x(xt, dt):      # [1024, 2048] -> [128, 8*2048] k-tiled
        return np.ascontiguousarray(
            xt.reshape(KT, 128, S).transpose(1, 0, 2).reshape(128, KT * S)
        ).astype(dt)

    xq_t = [tile_x(query[b].T, BF16_NP) for b in range(B)]
    xk_t = [tile_x(key[b].T, BF16_NP) for b in range(B)]
    xv_t = [tile_x(value[b].T, BF16_NP) for b in range(B)]

    def tile_w(WT, dt=BF16_NP):  # [1024, 256] -> [128, 8*256] k-tiled
        return np.ascontiguousarray(
            WT.reshape(KT, 128, DH).transpose(1, 0, 2).reshape(128, KT * DH)
        ).astype(dt)

    for c in range(N_CORES):
        b, hg = divmod(c, HG)
        hs = slice(hg * DH, (hg + 1) * DH)
        wo_tiled = np.ascontiguousarray(
            Wo[:, hs].T.reshape(2, 128, D_MODEL).transpose(1, 0, 2)
            .reshape(128, 2 * D_MODEL)).astype(BF16_NP)
        bqk_pack = np.concatenate(
            [bq[hs].reshape(2, 128).T, bk[hs].reshape(2, 128).T],
            axis=1)          # [128, 4] = bq cols | bk cols
        in_maps.append({
            "xq_t": xq_t[b],
            "xk_t": xk_t[b],
            "xv_t": xv_t[b],
            "wq_t": tile_w(Wq[hs, :].T),
            "wk_t": tile_w(Wk[hs, :].T),
            "wv_t": tile_w(Wv[hs, :].T),
            "wo_t": wo_tiled,
            "bqk": np.ascontiguousarray(bqk_pack),
            "bv": np.ascontiguousarray(bv[hs]).reshape(1, DH).astype(BF16_NP),
        })
    return in_maps


def run(inputs, trace=False, **spmd_kwargs):
    nc = _get_nc()
    in_maps = _make_in_maps(
        inputs["query"], inputs["key"], inputs["value"],
        inputs["Wq"], inputs["bq"], inputs["Wk"], inputs["bk"],
        inputs["Wv"], inputs["bv"], inputs["Wo"])
    res = run_bass_kernel_spmd(
        nc, in_maps, list(range(N_CORES)), trace=trace, **spmd_kwargs)
    bo = np.asarray(inputs["bo"], dtype=np.float32)
    out = np.empty((B, S, D_MODEL), dtype=np.float32)
    for b in range(B):
        acc = np.zeros((S, D_MODEL), dtype=np.float32)
        for hg in range(HG):
            acc += np.asarray(res.results[b * HG + hg]["y"], dtype=np.float32)
        out[b] = acc + bo
    return out, res


def kernel(**inputs) -> np.ndarray:
    out, _ = run(inputs, trace=False)
    return out


# revision 56
# speedup vs baseline: 1.0234x; 1.0130x over previous
"""MultiHeadAttention TRN2 Bass kernel (B=2, S=2048, D=1024, H=16, d=64).

Sharding: 8 cores = 2 (batch) x 4 (head groups of 4 heads), no collectives.
Each core computes, for its batch b and head slice hs (256 dims):
    K^T = (Wk[hs,:] @ x_k^T + bk)    [256, 2048]   (dh on partitions)
    Q^T likewise; V = x_v @ Wv[hs,:].T + bv        [2048, 256]  (s on partitions)
    per head pair (2m, 2m+1): S^T = K_h @ Q_h^T
    P^T = exp(S^T / 8)   (scores ~ N(0,1), exp is safe without max-sub)
    [O^T ; denom] = [V_h | 1]^T @ P^T   (ones column folds the softmax
                                         denominator into the PV matmul)
    O^T = O^T * (1/denom)
    y_partial = O^T.T @ Wo[:, hs].T     [2048, 1024]
Host: y[b] = sum of 4 head-group partials + bo.

Schedule: window pipeline paced by the Scalar exp floor (~135us) and the
serial PE matmul stream (~177us).  Window p emits scores of pair p while
pair p-1's PV drains three k behind (pv(p-1,k-3) follows scores(p,k), so
the window-boundary PSUM-acc eviction hides behind the first score
matmuls).  Softmax norm: denominators evicted to SBUF at window end, one
reciprocal_approx_fast (DVE custom op, ~5x faster than InstReciprocal),
a single K=2 PE matmul replicates both heads' recip rows across the 128
partitions, and one Pool-engine multiply applies it early next window.
V bias is folded into the PSUM eviction (Pool partition_broadcast of bv,
ones columns memset once) instead of a PE matmul per s-tile.  Head DMA:
window-0's exact operands (m=0 weight halves as strided slices + the
first x column blocks) go first, round-robined over the SP/ACT/Pool DGE
queues (each sustains ~95GB/s with queue depth 2).  The tail splits the
last pair into two 256-col halves, then half B's norm into two 128-col
pieces, pipelining each norm chain + yproj against the next PV stream;
tail evictions ride the post-exp idle Scalar engine (scalar.copy), and
y is written back in bf16 to halve the final DMA drain.
"""

import numpy as np
import ml_dtypes

import concourse.bass as bass
import concourse.tile as tile
import concourse.mybir as mybir
from concourse import bacc
from concourse.bass_utils import run_bass_kernel_spmd

D_MODEL = 1024
NUM_HEADS = 16
HEAD_DIM = 64
B, S = 2, 2048
N_CORES = 8
HG = 4                  # head-groups
HEADS_PER_CORE = NUM_HEADS // HG        # 4
DH = HEADS_PER_CORE * HEAD_DIM          # 256 output dims per core
KT = D_MODEL // 128                     # 8 contraction tiles
ST = S // 128                           # 16 sequence tiles
SB = S // 512                           # 4 sequence blocks of 512

F32 = mybir.dt.float32
F32R = mybir.dt.float32r
BF16 = mybir.dt.bfloat16
AF = mybir.ActivationFunctionType
BF16_NP = ml_dtypes.bfloat16

_cached_nc = None


def build_nc():
    nc = bacc.Bacc("TRN2", target_bir_lowering=False, debug=False)

    xq_t = nc.declare_dram_parameter("xq_t", [128, KT * S], BF16, isOutput=False)
    xk_t = nc.declare_dram_parameter("xk_t", [128, KT * S], BF16, isOutput=False)
    xv_t = nc.declare_dram_parameter("xv_t", [128, KT * S], BF16, isOutput=False)
    wq_t = nc.declare_dram_parameter("wq_t", [128, KT * DH], BF16, isOutput=False)
    wk_t = nc.declare_dram_parameter("wk_t", [128, KT * DH], BF16, isOutput=False)
    wv_t = nc.declare_dram_parameter("wv_t", [128, KT * DH], BF16, isOutput=False)
    wo_t = nc.declare_dram_parameter("wo_t", [128, 2 * D_MODEL], BF16, isOutput=False)
    bqk = nc.declare_dram_parameter("bqk", [128, 4], F32, isOutput=False)
    bv = nc.declare_dram_parameter("bv", [1, DH], BF16, isOutput=False)
    y = nc.declare_dram_parameter("y", [S, D_MODEL], BF16, isOutput=True)

    with tile.TileContext(nc) as tc:
        _emit(nc, tc, xq_t, xk_t, xv_t, wq_t, wk_t, wv_t, wo_t, bqk, bv, y)
    nc.compile()
    return nc


def _emit(nc, tc, xq_t, xk_t, xv_t, wq_t, wk_t, wv_t, wo_t, bqk, bv, y):
    from contextlib import ExitStack

    ctx = ExitStack()
    with ctx:
        # ---- persistent tiles -------------------------------------------
        persist = ctx.enter_context(tc.tile_pool(name="persist", bufs=1))
        qt = [persist.tile([128, S], BF16, tag=f"qt{m}", name=f"qt{m}")
              for m in range(2)]
        kt_sb = [persist.tile([128, S], BF16, tag=f"kt{m}", name=f"kt{m}")
                 for m in range(2)]
        v_sb = [persist.tile([128, HEADS_PER_CORE * 65], BF16, tag=f"v{i}",
                             name=f"v{i}") for i in range(ST)]
        ot = [persist.tile([128, S], BF16, tag=f"ot{m}", name=f"ot{m}")
              for m in range(2)]
        wo_flat = persist.tile([128, 2 * D_MODEL], BF16, tag="wof", name="wof")
        wo_r = [wo_flat[:, m * D_MODEL:(m + 1) * D_MODEL] for m in range(2)]
        ones2 = persist.tile([33, 128], F32, tag="ones2")
        ones2_r = persist.tile([33, 128], F32R, tag="ones2r")
        den2 = persist.tile([33, 512], F32, tag="den2")
        bqk_c = persist.tile([128, 4], F32, tag="bqk")  # bq|bk per-partition
        bq_c, bk_c = bqk_c[:, 0:2], bqk_c[:, 2:4]
        bv_r = persist.tile([1, DH], BF16, tag="bvr")
        bv_bc = persist.tile([128, DH], BF16, tag="bvbc")
        w_flat = {n: persist.tile([128, KT * DH], BF16,
                                  tag=f"w{n}", name=f"w{n}")
                  for n in ("k", "q", "v")}
        w3 = {n: w_flat[n].rearrange("p (k d) -> p k d", d=DH)
              for n in ("k", "q", "v")}
        x_flat = {n: persist.tile([128, KT * S], BF16,
                                  tag=f"x{n}", name=f"x{n}")
                  for n in ("k", "q", "v")}
        x3 = {n: x_flat[n].rearrange("p (k s) -> p k s", s=S)
              for n in ("k", "q", "v")}

        # ---- DMA (priority order == consumption order) ------------------
        # The lead is SP dispatch-rate bound (~650ns per dma_start), so the
        # first projections' exact needs go first: m=0 weight halves
        # (strided slice), then the x column blocks, spread across several
        # engines' DGE queues to overlap dispatch.
        def dma_cols(dst3, dram, c0, c1, eng=None, engs=None):
            dram3 = dram.rearrange("p (k s) -> p k s", s=S)
            for kp in range(KT // 2):
                e = engs[kp % len(engs)] if engs else (eng or nc.sync)
                e.dma_start(
                    dst3[:, 2 * kp:2 * kp + 2, c0:c1],
                    dram3[:, 2 * kp:2 * kp + 2, c0:c1])

        def dma_w_half(name, dram, m, eng=None):
            dram3 = dram.rearrange("p (k d) -> p k d", d=DH)
            (eng or nc.sync).dma_start(
                w3[name][:, :, m * 128:(m + 1) * 128],
                dram3[:, :, m * 128:(m + 1) * 128])

        def dma_w4(dst_tile, dram, n=4):
            w = dst_tile.shape[1] // n
            for j in range(n):
                nc.sync.dma_start(dst_tile[:, j * w:(j + 1) * w],
                                  dram[:, j * w:(j + 1) * w])

        # critical set for window 0, round-robined over the 3 DGE queues
        dma_w_half("k", wk_t, 0)                     # sync queue
        dma_cols(x3["k"], xk_t, 0, 512,
                 engs=[nc.scalar, nc.sync, nc.scalar, nc.gpsimd])
        dma_w_half("q", wq_t, 0, nc.gpsimd)          # pool queue
        dma_cols(x3["q"], xq_t, 0, 512,
                 engs=[nc.scalar, nc.sync, nc.scalar, nc.sync])
        dma_cols(x3["k"], xk_t, 512, 1024,
                 engs=[nc.gpsimd, nc.scalar, nc.sync, nc.scalar])
        nc.gpsimd.dma_start(bqk_c[:], bqk[:, :])
        dma_w_half("k", wk_t, 1, nc.gpsimd)
        dma_cols(x3["k"], xk_t, 1024, 1536, nc.scalar)
        dma_w_half("q", wq_t, 1)
        dma_cols(x3["q"], xq_t, 512, 1024, nc.gpsimd)
        dma_cols(x3["k"], xk_t, 1536, S, nc.scalar)
        dma_cols(x3["q"], xq_t, 1024, S)
        dma_w_half("v", wv_t, 0)
        dma_w_half("v", wv_t, 1)
        nc.sync.dma_start(bv_r[:], bv[:])
        dma_cols(x3["v"], xv_t, 0, S)
        dma_w4(wo_flat, wo_t)

        # ---- pipelined-body pools ---------------------------------------
        ps_s = ctx.enter_context(
            tc.tile_pool(name="pss", bufs=2, space="PSUM"))      # 4 banks
        ps_acc = ctx.enter_context(
            tc.tile_pool(name="psacc", bufs=1, space="PSUM"))    # 2 banks
        ps_w = ctx.enter_context(
            tc.tile_pool(name="psw", bufs=2, space="PSUM"))      # 2 banks
        pt_pool = ctx.enter_context(tc.tile_pool(name="pt", bufs=19))
        sm_pool = ctx.enter_context(tc.tile_pool(name="small", bufs=1))
        sm2_pool = ctx.enter_context(tc.tile_pool(name="small2", bufs=2))
        y_pool = ctx.enter_context(tc.tile_pool(name="ysb", bufs=2))

        # constants: ones2_r replicate pattern (partition 0 -> out
        # partitions 0-63, partition 32 -> 64-127; the rest zero),
        # den2 init so unused rows are finite, broadcast bv, v_sb ones
        nc.vector.memset(ones2[:], 0.0)
        nc.vector.memset(ones2[0:1, 0:64], 1.0)
        nc.vector.memset(ones2[32:33, 64:128], 1.0)
        nc.vector.tensor_copy(ones2_r[:], ones2[:])
        nc.vector.memset(den2[:], 1.0)
        nc.gpsimd.partition_broadcast(bv_bc[:], bv_r[:])
        for i in range(ST):
            vv = v_sb[i].rearrange("p (h c) -> p h c", c=65)
            nc.gpsimd.memset(vv[:, :, 64], 1.0)

        # ---- building blocks --------------------------------------------
        def proj_qk_m(name, dst, bias_c, nb, m):
            """Project one (512-col, m-half) block of Q^T or K^T (bf16)."""
            ps = ps_w.tile([128, 512], F32, tag="pw", name="pw")
            for k in range(KT):
                nc.tensor.matmul(
                    ps[:],
                    w3[name][:, k, m * 128:(m + 1) * 128],
                    x3[name][:, k, nb * 512:(nb + 1) * 512],
                    start=(k == 0), stop=(k == KT - 1),
                )
            nc.vector.tensor_scalar_add(
                dst[m][:, nb * 512:(nb + 1) * 512], ps[:],
                bias_c[:, m:m + 1])

        def v_chunk(i):
            """Project V for s-tile i into v_sb[i]; bias folded into the
            DVE eviction (bv broadcast tile), ones column pre-set."""
            ps = ps_w.tile([128, 512], F32, tag="pw", name="pw")
            for k in range(KT):
                nc.tensor.matmul(
                    ps[:, 0:256],
                    x3["v"][:, k, i * 128:(i + 1) * 128],
                    w3["v"][:, k, :],
                    start=(k == 0), stop=(k == KT - 1),
                )
            src = ps[:, 0:256].rearrange("p (h c) -> p h c", c=64)
            vv = v_sb[i].rearrange("p (h c) -> p h c", c=65)
            bvv = bv_bc.rearrange("p (h c) -> p h c", c=64)
            nc.vector.tensor_add(vv[:, :, 0:64], src, bvv)

        def scores(qb, m, k):
            """Score pair (heads 2m,2m+1), sk-tile k, sq-block qb."""
            ss = ps_s.tile([128, 1024], F32, tag="ss", name="ss")
            for p2 in range(2):
                po = 64 * p2
                nc.tensor.matmul(
                    ss[:, p2 * 512:(p2 + 1) * 512],
                    kt_sb[m][po:po + 64, k * 128:(k + 1) * 128],
                    qt[m][po:po + 64, qb * 512:(qb + 1) * 512],
                    start=True, stop=True,
                )
            pt = pt_pool.tile([128, 1024], BF16, tag="pt", name="pt")
            nc.scalar.activation(
                pt[:], ss[:], AF.Exp, scale=1.0 / float(np.sqrt(HEAD_DIM)))
            return pt

        def pv(m, k, pt, accs, c0=0, c1=512, start=None, stop=None):
            """PV for both heads of pair-half m over pt columns [c0:c1)."""
            for p2 in range(2):
                h = 2 * m + p2
                nc.tensor.matmul(
                    accs[p2][:, 0:c1 - c0],
                    v_sb[k][:, h * 65:(h + 1) * 65],
                    pt[:, p2 * 512 + c0:p2 * 512 + c1],
                    start=(k == 0) if start is None else start,
                    stop=(k == ST - 1) if stop is None else stop,
                )

        def norm_stage1(accs, w=512, c0=0, act_evict=False):
            """Evict O rows + denominators to SBUF (frees the PSUM accs)
            and compute the batched approx reciprocal on DVE.  With
            act_evict the O-row copies ride the (post-exp idle) Scalar
            engine instead of the congested DVE queue."""
            osb = sm2_pool.tile([128, 512], BF16, tag="osb", name="osb")
            recip2 = sm_pool.tile([33, 512], F32, tag="recipf", name="recipf")
            recip2_r = sm2_pool.tile([33, 512], F32R, tag="recip2",
                                     name="recip2")
            # denominators first: the recip chain starts ~1.4us earlier,
            # which is what the tail (and the next window's apply) wait on
            for p2 in range(2):
                nc.vector.tensor_copy(den2[32 * p2:32 * p2 + 1, 0:w],
                                      accs[p2][64:65, c0:c0 + w])
            with nc.allow_low_precision(reason="softmax denom"):
                nc.vector.reciprocal_approx_fast(recip2[:, 0:w], den2[:, 0:w])
                nc.vector.tensor_copy(recip2_r[:, 0:w], recip2[:, 0:w])
            for p2 in range(2):
                if act_evict:
                    nc.scalar.copy(osb[64 * p2:64 * p2 + 64, 0:w],
                                   accs[p2][0:64, c0:c0 + w])
                else:
                    nc.vector.tensor_copy(osb[64 * p2:64 * p2 + 64, 0:w],
                                          accs[p2][0:64, c0:c0 + w])
            return (osb, recip2_r)

        def norm_apply(qb, m, st, oc=0, w=512):
            """ot[m][:, qb block cols oc:oc+w] = O^T * recip: one K=2 PE
            matmul replicates both heads' recip rows; one Pool multiply."""
            osb, recip2 = st
            rep = ps_w.tile([128, 512], F32, tag="pw", name="pw")
            nc.tensor.matmul(
                rep[:, 0:w], ones2_r[:],
                recip2[:, 0:w],
                start=True, stop=True,
            )
            rep_sb = sm_pool.tile([128, 512], BF16, tag="repsb", name="repsb")
            nc.vector.tensor_copy(rep_sb[:, 0:w], rep[:, 0:w])
            nc.gpsimd.tensor_mul(
                ot[m][:, qb * 512 + oc:qb * 512 + oc + w],
                osb[:, 0:w], rep_sb[:, 0:w])

        def yproj_i(i, ysb_holder, eng=None, act_evict=False):
            """Output projection for s-tile i; DMA per half as it lands."""
            if ysb_holder[0] is None:
                ysb_holder[0] = y_pool.tile([128, D_MODEL], BF16, tag="ysb",
                                            name="ysb")
            ysb = ysb_holder[0]
            for nb2 in range(2):
                ps = ps_w.tile([128, 512], F32, tag="pw", name="pw")
                for m in range(2):
                    nc.tensor.matmul(
                        ps[:],
                        ot[m][:, i * 128:(i + 1) * 128],
                        wo_r[m][:, nb2 * 512:(nb2 + 1) * 512],
                        start=(m == 0), stop=(m == 1),
                    )
                if act_evict:
                    nc.scalar.copy(ysb[:, nb2 * 512:(nb2 + 1) * 512], ps[:])
                else:
                    nc.vector.tensor_copy(
                        ysb[:, nb2 * 512:(nb2 + 1) * 512], ps[:])
                (eng or nc.sync).dma_start(
                    y[i * 128:(i + 1) * 128, nb2 * 512:(nb2 + 1) * 512],
                    ysb[:, nb2 * 512:(nb2 + 1) * 512])
            ysb_holder[0] = None

        # =============== emission schedule ===============================
        # Window p: scores(p, k) leads pv(p-1, k-1) by one k so the
        # boundary acc eviction hides behind the first score matmul.
        pairs = [(qb, m) for qb in range(SB) for m in range(2)]
        yh = [None]

        def alloc_accs():
            return [ps_acc.tile([65, 512], F32, tag=f"acc{pp}",
                                name=f"acc{pp}") for pp in range(2)]

        proj_slots = {
            (0, 1): ("k", 0, 1),   # kt m0 nb1, read from scores(0,4)
            (0, 3): ("k", 1, 0),   # window-1 operands
            (0, 5): ("q", 1, 0),
            (0, 6): ("k", 0, 2),   # read from scores(0,8)
            (0, 7): ("k", 1, 1),
            (0, 9): ("k", 0, 3),   # read from scores(0,12)
            (0, 10): ("k", 1, 2),
            (0, 12): ("k", 1, 3),
            (0, 11): ("q", 0, 1),  # window-2 operand
            (0, 13): ("q", 1, 1),
            (2, 6): ("q", 0, 2), (2, 11): ("q", 1, 2),
            (4, 6): ("q", 0, 3), (4, 11): ("q", 1, 3),
        }
        yproj_slots = {
            (3, 6): 0, (3, 9): 1, (3, 12): 2, (4, 2): 3,     # yproj(0)
            (5, 6): 4, (5, 9): 5, (5, 12): 6, (6, 2): 7,     # yproj(1)
            (7, 6): 8, (7, 9): 9, (7, 12): 10, (7, 14): 11,  # yproj(2)
        }

        # lead-in: K and Q m=0 of block 0 only (the m=1 halves are
        # window-0 slots), so the first matmul starts on minimal DMA.
        proj_qk_m("k", kt_sb, bk_c, 0, 0)
        proj_qk_m("q", qt, bq_c, 0, 0)

        pts_prev = None
        accs_run = None
        apply_q = []            # FIFO of (qb, m, stage1 state)
        for p in range(len(pairs)):
            qb, m = pairs[p]
            if p >= 1:
                accs_run = alloc_accs()
            pts_cur = []
            for k in range(ST):
                pts_cur.append(scores(qb, m, k))
                if k == 6 and apply_q:
                    norm_apply(*apply_q.pop(0))
                if p == 1:
                    v_chunk(k)
                if p >= 1 and k >= 3:
                    pv(pairs[p - 1][1], k - 3, pts_prev[k - 3], accs_run)
                if (p, k) in proj_slots:
                    nm, pm, pnb = proj_slots[(p, k)]
                    proj_qk_m(nm, kt_sb if nm == "k" else qt,
                              bk_c if nm == "k" else bq_c, pnb, pm)
                if (p, k) in yproj_slots:
                    yproj_i(yproj_slots[(p, k)], yh)
            if p >= 1:
                for kk in range(ST - 3, ST):
                    pv(pairs[p - 1][1], kk, pts_prev[kk], accs_run)
                st = norm_stage1(accs_run,
                                 act_evict=(p == len(pairs) - 1))
                apply_q.append((pairs[p - 1][0], pairs[p - 1][1], st))
            pts_prev = pts_cur

        # ---- tail: PV + norm of the last pair (3,1), split into two
        # 256-col halves so half A's norm chain + yproj(12,13) overlap
        # half B's PV stream.
        qb_l, m_l = pairs[-1]
        accsA = alloc_accs()
        for k in range(ST):
            pv(m_l, k, pts_prev[k], accsA, c0=0, c1=256)
            if k == 5 and apply_q:
                norm_apply(*apply_q.pop(0))
        stA = norm_stage1(accsA, w=256, act_evict=True)

        ssb = ps_s.tile([128, 1024], F32, tag="ss", name="ss")
        accsB = [ssb[0:65, 0:256], ssb[0:65, 512:768]]
        for k in range(ST):
            pv(m_l, k, pts_prev[k], accsB, c0=256, c1=512)
            if k == 3:
                norm_apply(qb_l, m_l, stA, oc=0, w=256)
            if k == 6:
                yproj_i(4 * qb_l + 0, yh, act_evict=True)
            if k == 11:
                yproj_i(4 * qb_l + 1, yh, act_evict=True)
        # endgame: two 128-col pieces of half B so yproj(14) starts
        # while piece 2's norm chain is still in flight
        stB1 = norm_stage1(accsB, w=128, c0=0, act_evict=True)
        norm_apply(qb_l, m_l, stB1, oc=256, w=128)
        stB2 = norm_stage1(accsB, w=128, c0=128, act_evict=True)
        yproj_i(4 * qb_l + 2, yh, act_evict=True)
        norm_apply(qb_l, m_l, stB2, oc=384, w=128)
        yproj_i(4 * qb_l + 3, yh, act_evict=True)


def _get_nc():
    global _cached_nc
    if _cached_nc is None:
        _cached_nc = build_nc()
    return _cached_nc


def _make_in_maps(query, key, value, Wq, bq, Wk, bk, Wv, bv, Wo):
    """Shard + transpose + bf16-cast on host: core c = (b, hg), b = c // HG."""
    query = np.asarray(query, dtype=np.float32)
    key = np.asarray(key, dtype=np.float32)
    value = np.asarray(value, dtype=np.float32)
    Wq, Wk, Wv, Wo = (np.asarray(w, dtype=np.float32) for w in (Wq, Wk, Wv, Wo))
    bq, bk, bv = (np.asarray(b_, dtype=np.float32) for b_ in (bq, bk, bv))
    in_maps = []

    def tile_x(xt, dt):      # [1024, 2048] -> [128, 8*2048] k-tiled
        return np.ascontiguousarray(
            xt.reshape(KT, 128, S).transpose(1, 0, 2).reshape(128, KT * S)
        ).astype(dt)

    xq_t = [tile_x(query[b].T, BF16_NP) for b in range(B)]
    xk_t = [tile_x(key[b].T, BF16_NP) for b in range(B)]
    xv_t = [tile_x(value[b].T, BF16_NP) for b in range(B)]

    def tile_w(WT, dt=BF16_NP):  # [1024, 256] -> [128, 8*256] k-tiled
        return np.ascontiguousarray(
            WT.reshape(KT, 128, DH).transpose(1, 0, 2).reshape(128, KT * DH)
        ).astype(dt)

    for c in range(N_CORES):
        b, hg = divmod(c, HG)
        hs = slice(hg * DH, (hg + 1) * DH)
        wo_tiled = np.ascontiguousarray(
            Wo[:, hs].T.reshape(2, 128, D_MODEL).transpose(1, 0, 2)
            .reshape(128, 2 * D_MODEL)).astype(BF16_NP)
        bqk_pack = np.concatenate(
            [bq[hs].reshape(2, 128).T, bk[hs].reshape(2, 128).T],
            axis=1)          # [128, 4] = bq cols | bk cols
        in_maps.append({
            "xq_t": xq_t[b],
            "xk_t": xk_t[b],
            "xv_t": xv_t[b],
            "wq_t": tile_w(Wq[hs, :].T),
            "wk_t": tile_w(Wk[hs, :].T),
            "wv_t": tile_w(Wv[hs, :].T),
            "wo_t": wo_tiled,
            "bqk": np.ascontiguousarray(bqk_pack),
            "bv": np.ascontiguousarray(bv[hs]).reshape(1, DH).astype(BF16_NP),
        })
    return in_maps


def run(inputs, trace=False, **spmd_kwargs):
    nc = _get_nc()
    in_maps = _make_in_maps(
        inputs["query"], inputs["key"], inputs["value"],
        inputs["Wq"], inputs["bq"], inputs["Wk"], inputs["bk"],
        inputs["Wv"], inputs["bv"], inputs["Wo"])
    res = run_bass_kernel_spmd(
        nc, in_maps, list(range(N_CORES)), trace=trace, **spmd_kwargs)
    bo = np.asarray(inputs["bo"], dtype=np.float32)
    out = np.empty((B, S, D_MODEL), dtype=np.float32)
    for b in range(B):
        acc = np.zeros((S, D_MODEL), dtype=np.float32)
        for hg in range(HG):
            acc += np.asarray(res.results[b * HG + hg]["y"], dtype=np.float32)
        out[b] = acc + bo
    return out, res


def kernel(**inputs) -> np.ndarray:
    out, _ = run(inputs, trace=False)
    return out


# revision 57
# speedup vs baseline: 1.0305x; 1.0069x over previous
"""MultiHeadAttention TRN2 Bass kernel (B=2, S=2048, D=1024, H=16, d=64).

Sharding: 8 cores = 2 (batch) x 4 (head groups of 4 heads), no collectives.
Each core computes, for its batch b and head slice hs (256 dims):
    K^T = (Wk[hs,:] @ x_k^T + bk)    [256, 2048]   (dh on partitions)
    Q^T likewise; V = x_v @ Wv[hs,:].T + bv        [2048, 256]  (s on partitions)
    per head pair (2m, 2m+1): S^T = K_h @ Q_h^T
    P^T = exp(S^T / 8)   (scores ~ N(0,1), exp is safe without max-sub)
    [O^T ; denom] = [V_h | 1]^T @ P^T   (ones column folds the softmax
                                         denominator into the PV matmul)
    O^T = O^T * (1/denom)
    y_partial = O^T.T @ Wo[:, hs].T     [2048, 1024]
Host: y[b] = sum of 4 head-group partials + bo.

Schedule: window pipeline paced by the Scalar exp floor (~135us) and the
serial PE matmul stream (~177us).  Window p emits scores of pair p while
pair p-1's PV drains three k behind (pv(p-1,k-3) follows scores(p,k), so
the window-boundary PSUM-acc eviction hides behind the first score
matmuls).  Softmax norm: denominators evicted to SBUF at window end, one
reciprocal_approx_fast (DVE custom op, ~5x faster than InstReciprocal),
a single K=2 PE matmul replicates both heads' recip rows across the 128
partitions, and one Pool-engine multiply applies it early next window.
V bias is folded into the PSUM eviction (Pool partition_broadcast of bv,
ones columns memset once) instead of a PE matmul per s-tile.  Head DMA:
window-0's exact operands (m=0 weight halves as strided slices + the
first x column blocks) go first, round-robined over the SP/ACT/Pool DGE
queues (each sustains ~95GB/s with queue depth 2).  The tail splits the
last pair into two 256-col halves, then half B's norm into two 128-col
pieces, pipelining each norm chain + yproj against the next PV stream;
tail evictions ride the post-exp idle Scalar engine (scalar.copy), and
y is written back in bf16 to halve the final DMA drain.
"""

import numpy as np
import ml_dtypes

import concourse.bass as bass
import concourse.tile as tile
import concourse.mybir as mybir
from concourse import bacc
from concourse.bass_utils import run_bass_kernel_spmd

D_MODEL = 1024
NUM_HEADS = 16
HEAD_DIM = 64
B, S = 2, 2048
N_CORES = 8
HG = 4                  # head-groups
HEADS_PER_CORE = NUM_HEADS // HG        # 4
DH = HEADS_PER_CORE * HEAD_DIM          # 256 output dims per core
KT = D_MODEL // 128                     # 8 contraction tiles
ST = S // 128                           # 16 sequence tiles
SB = S // 512                           # 4 sequence blocks of 512

F32 = mybir.dt.float32
F32R = mybir.dt.float32r
BF16 = mybir.dt.bfloat16
AF = mybir.ActivationFunctionType
BF16_NP = ml_dtypes.bfloat16

_cached_nc = None


def build_nc():
    nc = bacc.Bacc("TRN2", target_bir_lowering=False, debug=False)

    xq_t = nc.declare_dram_parameter("xq_t", [128, KT * S], BF16, isOutput=False)
    xk_t = nc.declare_dram_parameter("xk_t", [128, KT * S], BF16, isOutput=False)
    xv_t = nc.declare_dram_parameter("xv_t", [128, KT * S], BF16, isOutput=False)
    wq_t = nc.declare_dram_parameter("wq_t", [128, KT * DH], BF16, isOutput=False)
    wk_t = nc.declare_dram_parameter("wk_t", [128, KT * DH], BF16, isOutput=False)
    wv_t = nc.declare_dram_parameter("wv_t", [128, KT * DH], BF16, isOutput=False)
    wo_t = nc.declare_dram_parameter("wo_t", [128, 2 * D_MODEL], BF16, isOutput=False)
    bqk = nc.declare_dram_parameter("bqk", [128, 4], F32, isOutput=False)
    bv = nc.declare_dram_parameter("bv", [1, DH], BF16, isOutput=False)
    y = nc.declare_dram_parameter("y", [S, D_MODEL], BF16, isOutput=True)

    with tile.TileContext(nc) as tc:
        _emit(nc, tc, xq_t, xk_t, xv_t, wq_t, wk_t, wv_t, wo_t, bqk, bv, y)
    nc.compile()
    return nc


def _emit(nc, tc, xq_t, xk_t, xv_t, wq_t, wk_t, wv_t, wo_t, bqk, bv, y):
    from contextlib import ExitStack

    ctx = ExitStack()
    with ctx:
        # ---- persistent tiles -------------------------------------------
        persist = ctx.enter_context(tc.tile_pool(name="persist", bufs=1))
        qt = [persist.tile([128, S], BF16, tag=f"qt{m}", name=f"qt{m}")
              for m in range(2)]
        kt_sb = [persist.tile([128, S], BF16, tag=f"kt{m}", name=f"kt{m}")
                 for m in range(2)]
        v_sb = [persist.tile([128, HEADS_PER_CORE * 65], BF16, tag=f"v{i}",
                             name=f"v{i}") for i in range(ST)]
        ot = [persist.tile([128, S], BF16, tag=f"ot{m}", name=f"ot{m}")
              for m in range(2)]
        wo_flat = persist.tile([128, 2 * D_MODEL], BF16, tag="wof", name="wof")
        wo_r = [wo_flat[:, m * D_MODEL:(m + 1) * D_MODEL] for m in range(2)]
        ones2 = persist.tile([33, 128], F32, tag="ones2")
        ones2_r = persist.tile([33, 128], F32R, tag="ones2r")
        den2 = persist.tile([33, 512], F32, tag="den2")
        bqk_c = persist.tile([128, 4], F32, tag="bqk")  # bq|bk per-partition
        bq_c, bk_c = bqk_c[:, 0:2], bqk_c[:, 2:4]
        bv_r = persist.tile([1, DH], BF16, tag="bvr")
        bv_bc = persist.tile([128, DH], BF16, tag="bvbc")
        w_flat = {n: persist.tile([128, KT * DH], BF16,
                                  tag=f"w{n}", name=f"w{n}")
                  for n in ("k", "q", "v")}
        w3 = {n: w_flat[n].rearrange("p (k d) -> p k d", d=DH)
              for n in ("k", "q", "v")}
        x_flat = {n: persist.tile([128, KT * S], BF16,
                                  tag=f"x{n}", name=f"x{n}")
                  for n in ("k", "q", "v")}
        x3 = {n: x_flat[n].rearrange("p (k s) -> p k s", s=S)
              for n in ("k", "q", "v")}

        # ---- DMA (priority order == consumption order) ------------------
        # The lead is SP dispatch-rate bound (~650ns per dma_start), so the
        # first projections' exact needs go first: m=0 weight halves
        # (strided slice), then the x column blocks, spread across several
        # engines' DGE queues to overlap dispatch.
        def dma_cols(dst3, dram, c0, c1, eng=None, engs=None):
            dram3 = dram.rearrange("p (k s) -> p k s", s=S)
            for kp in range(KT // 2):
                e = engs[kp % len(engs)] if engs else (eng or nc.sync)
                e.dma_start(
                    dst3[:, 2 * kp:2 * kp + 2, c0:c1],
                    dram3[:, 2 * kp:2 * kp + 2, c0:c1])

        def dma_w_half(name, dram, m, eng=None):
            dram3 = dram.rearrange("p (k d) -> p k d", d=DH)
            (eng or nc.sync).dma_start(
                w3[name][:, :, m * 128:(m + 1) * 128],
                dram3[:, :, m * 128:(m + 1) * 128])

        def dma_w4(dst_tile, dram, n=4):
            w = dst_tile.shape[1] // n
            for j in range(n):
                nc.sync.dma_start(dst_tile[:, j * w:(j + 1) * w],
                                  dram[:, j * w:(j + 1) * w])

        # critical set for window 0, round-robined over the 3 DGE queues
        dma_w_half("k", wk_t, 0)                     # sync queue
        dma_cols(x3["k"], xk_t, 0, 512,
                 engs=[nc.scalar, nc.sync, nc.scalar, nc.gpsimd])
        dma_w_half("q", wq_t, 0, nc.gpsimd)          # pool queue
        dma_cols(x3["q"], xq_t, 0, 512,
                 engs=[nc.scalar, nc.sync, nc.scalar, nc.sync])
        dma_cols(x3["k"], xk_t, 512, 1024,
                 engs=[nc.gpsimd, nc.scalar, nc.sync, nc.scalar])
        nc.gpsimd.dma_start(bqk_c[:], bqk[:, :])
        dma_w_half("k", wk_t, 1, nc.gpsimd)
        dma_cols(x3["k"], xk_t, 1024, 1536, nc.scalar)
        dma_w_half("q", wq_t, 1)
        dma_cols(x3["q"], xq_t, 512, 1024, nc.gpsimd)
        dma_cols(x3["k"], xk_t, 1536, S, nc.scalar)
        dma_cols(x3["q"], xq_t, 1024, S)
        dma_w_half("v", wv_t, 0)
        dma_w_half("v", wv_t, 1)
        nc.sync.dma_start(bv_r[:], bv[:])
        dma_cols(x3["v"], xv_t, 0, S)
        dma_w4(wo_flat, wo_t)

        # ---- pipelined-body pools ---------------------------------------
        ps_s = ctx.enter_context(
            tc.tile_pool(name="pss", bufs=2, space="PSUM"))      # 4 banks
        ps_acc = ctx.enter_context(
            tc.tile_pool(name="psacc", bufs=1, space="PSUM"))    # 2 banks
        ps_w = ctx.enter_context(
            tc.tile_pool(name="psw", bufs=2, space="PSUM"))      # 2 banks
        pt_pool = ctx.enter_context(tc.tile_pool(name="pt", bufs=19))
        sm_pool = ctx.enter_context(tc.tile_pool(name="small", bufs=1))
        sm2_pool = ctx.enter_context(tc.tile_pool(name="small2", bufs=2))
        y_pool = ctx.enter_context(tc.tile_pool(name="ysb", bufs=2))

        # constants: ones2_r replicate pattern (partition 0 -> out
        # partitions 0-63, partition 32 -> 64-127; the rest zero),
        # den2 init so unused rows are finite, broadcast bv, v_sb ones
        nc.vector.memset(ones2[:], 0.0)
        nc.vector.memset(ones2[0:1, 0:64], 1.0)
        nc.vector.memset(ones2[32:33, 64:128], 1.0)
        nc.vector.tensor_copy(ones2_r[:], ones2[:])
        nc.vector.memset(den2[:], 1.0)
        nc.gpsimd.partition_broadcast(bv_bc[:], bv_r[:])
        for i in range(ST):
            vv = v_sb[i].rearrange("p (h c) -> p h c", c=65)
            nc.gpsimd.memset(vv[:, :, 64], 1.0)

        # ---- building blocks --------------------------------------------
        def proj_qk_m(name, dst, bias_c, nb, m):
            """Project one (512-col, m-half) block of Q^T or K^T (bf16)."""
            ps = ps_w.tile([128, 512], F32, tag="pw", name="pw")
            for k in range(KT):
                nc.tensor.matmul(
                    ps[:],
                    w3[name][:, k, m * 128:(m + 1) * 128],
                    x3[name][:, k, nb * 512:(nb + 1) * 512],
                    start=(k == 0), stop=(k == KT - 1),
                )
            nc.vector.tensor_scalar_add(
                dst[m][:, nb * 512:(nb + 1) * 512], ps[:],
                bias_c[:, m:m + 1])

        def v_chunk(i):
            """Project V for s-tile i into v_sb[i]; bias folded into the
            DVE eviction (bv broadcast tile), ones column pre-set."""
            ps = ps_w.tile([128, 512], F32, tag="pw", name="pw")
            for k in range(KT):
                nc.tensor.matmul(
                    ps[:, 0:256],
                    x3["v"][:, k, i * 128:(i + 1) * 128],
                    w3["v"][:, k, :],
                    start=(k == 0), stop=(k == KT - 1),
                )
            src = ps[:, 0:256].rearrange("p (h c) -> p h c", c=64)
            vv = v_sb[i].rearrange("p (h c) -> p h c", c=65)
            bvv = bv_bc.rearrange("p (h c) -> p h c", c=64)
            nc.vector.tensor_add(vv[:, :, 0:64], src, bvv)

        def scores(qb, m, k):
            """Score pair (heads 2m,2m+1), sk-tile k, sq-block qb."""
            ss = ps_s.tile([128, 1024], F32, tag="ss", name="ss")
            for p2 in range(2):
                po = 64 * p2
                nc.tensor.matmul(
                    ss[:, p2 * 512:(p2 + 1) * 512],
                    kt_sb[m][po:po + 64, k * 128:(k + 1) * 128],
                    qt[m][po:po + 64, qb * 512:(qb + 1) * 512],
                    start=True, stop=True,
                )
            pt = pt_pool.tile([128, 1024], BF16, tag="pt", name="pt")
            nc.scalar.activation(
                pt[:], ss[:], AF.Exp, scale=1.0 / float(np.sqrt(HEAD_DIM)))
            return pt

        def pv(m, k, pt, accs, c0=0, c1=512, start=None, stop=None):
            """PV for both heads of pair-half m over pt columns [c0:c1)."""
            for p2 in range(2):
                h = 2 * m + p2
                nc.tensor.matmul(
                    accs[p2][:, 0:c1 - c0],
                    v_sb[k][:, h * 65:(h + 1) * 65],
                    pt[:, p2 * 512 + c0:p2 * 512 + c1],
                    start=(k == 0) if start is None else start,
                    stop=(k == ST - 1) if stop is None else stop,
                )

        def norm_stage1(accs, w=512, c0=0, act_evict=False):
            """Evict O rows + denominators to SBUF (frees the PSUM accs)
            and compute the batched approx reciprocal on DVE.  With
            act_evict the O-row copies ride the (post-exp idle) Scalar
            engine instead of the congested DVE queue."""
            osb = sm2_pool.tile([128, 512], BF16, tag="osb", name="osb")
            recip2 = sm_pool.tile([33, 512], F32, tag="recipf", name="recipf")
            recip2_r = sm2_pool.tile([33, 512], F32R, tag="recip2",
                                     name="recip2")
            # denominators first: the recip chain starts ~1.4us earlier,
            # which is what the tail (and the next window's apply) wait on
            for p2 in range(2):
                nc.vector.tensor_copy(den2[32 * p2:32 * p2 + 1, 0:w],
                                      accs[p2][64:65, c0:c0 + w])
            with nc.allow_low_precision(reason="softmax denom"):
                nc.vector.reciprocal_approx_fast(recip2[:, 0:w], den2[:, 0:w])
                nc.vector.tensor_copy(recip2_r[:, 0:w], recip2[:, 0:w])
            for p2 in range(2):
                if act_evict:
                    nc.scalar.copy(osb[64 * p2:64 * p2 + 64, 0:w],
                                   accs[p2][0:64, c0:c0 + w])
                else:
                    nc.vector.tensor_copy(osb[64 * p2:64 * p2 + 64, 0:w],
                                          accs[p2][0:64, c0:c0 + w])
            return (osb, recip2_r)

        def norm_apply(qb, m, st, oc=0, w=512):
            """ot[m][:, qb block cols oc:oc+w] = O^T * recip: one K=2 PE
            matmul replicates both heads' recip rows; one Pool multiply."""
            osb, recip2 = st
            rep = ps_w.tile([128, 512], F32, tag="pw", name="pw")
            nc.tensor.matmul(
                rep[:, 0:w], ones2_r[:],
                recip2[:, 0:w],
                start=True, stop=True,
            )
            rep_sb = sm_pool.tile([128, 512], BF16, tag="repsb", name="repsb")
            nc.vector.tensor_copy(rep_sb[:, 0:w], rep[:, 0:w])
            nc.gpsimd.tensor_mul(
                ot[m][:, qb * 512 + oc:qb * 512 + oc + w],
                osb[:, 0:w], rep_sb[:, 0:w])

        def yproj_i(i, ysb_holder, eng=None, act_evict=False):
            """Output projection for s-tile i; DMA per half as it lands."""
            if ysb_holder[0] is None:
                ysb_holder[0] = y_pool.tile([128, D_MODEL], BF16, tag="ysb",
                                            name="ysb")
            ysb = ysb_holder[0]
            for nb2 in range(2):
                ps = ps_w.tile([128, 512], F32, tag="pw", name="pw")
                for m in range(2):
                    nc.tensor.matmul(
                        ps[:],
                        ot[m][:, i * 128:(i + 1) * 128],
                        wo_r[m][:, nb2 * 512:(nb2 + 1) * 512],
                        start=(m == 0), stop=(m == 1),
                    )
                if act_evict:
                    nc.scalar.copy(ysb[:, nb2 * 512:(nb2 + 1) * 512], ps[:])
                else:
                    nc.vector.tensor_copy(
                        ysb[:, nb2 * 512:(nb2 + 1) * 512], ps[:])
                (eng or nc.sync).dma_start(
                    y[i * 128:(i + 1) * 128, nb2 * 512:(nb2 + 1) * 512],
                    ysb[:, nb2 * 512:(nb2 + 1) * 512])
            ysb_holder[0] = None

        # =============== emission schedule ===============================
        # Window p: scores(p, k) leads pv(p-1, k-1) by one k so the
        # boundary acc eviction hides behind the first score matmul.
        pairs = [(qb, m) for qb in range(SB) for m in range(2)]
        yh = [None]

        def alloc_accs():
            return [ps_acc.tile([65, 512], F32, tag=f"acc{pp}",
                                name=f"acc{pp}") for pp in range(2)]

        proj_slots = {
            (0, 1): ("k", 0, 1),   # kt m0 nb1, read from scores(0,4)
            (0, 3): ("k", 1, 0),   # window-1 operands
            (0, 5): ("q", 1, 0),
            (0, 6): ("k", 0, 2),   # read from scores(0,8)
            (0, 7): ("k", 1, 1),
            (0, 9): ("k", 0, 3),   # read from scores(0,12)
            (0, 10): ("k", 1, 2),
            (0, 12): ("k", 1, 3),
            (0, 11): ("q", 0, 1),  # window-2 operand
            (0, 13): ("q", 1, 1),
            (2, 6): ("q", 0, 2), (2, 11): ("q", 1, 2),
            (4, 6): ("q", 0, 3), (4, 11): ("q", 1, 3),
        }
        # yproj slots start at k=9: the block's second apply lands at k=6
        # and its Pool multiply only completes ~k=8.5, so earlier slots
        # stall the PE on the freshly-applied ot block
        yproj_slots = {
            (3, 9): 0, (3, 11): 1, (3, 13): 2, (4, 2): 3,    # yproj(0)
            (5, 9): 4, (5, 11): 5, (5, 13): 6, (6, 2): 7,    # yproj(1)
            (7, 9): 8, (7, 11): 9, (7, 13): 10, (7, 14): 11, # yproj(2)
        }

        # lead-in: K and Q m=0 of block 0 only (the m=1 halves are
        # window-0 slots), so the first matmul starts on minimal DMA.
        proj_qk_m("k", kt_sb, bk_c, 0, 0)
        proj_qk_m("q", qt, bq_c, 0, 0)

        pts_prev = None
        accs_run = None
        apply_q = []            # FIFO of (qb, m, stage1 state)
        for p in range(len(pairs)):
            qb, m = pairs[p]
            if p >= 1:
                accs_run = alloc_accs()
            pts_cur = []
            for k in range(ST):
                pts_cur.append(scores(qb, m, k))
                if k == 6 and apply_q:
                    norm_apply(*apply_q.pop(0))
                if p == 1:
                    v_chunk(k)
                if p >= 1 and k >= 3:
                    pv(pairs[p - 1][1], k - 3, pts_prev[k - 3], accs_run)
                if (p, k) in proj_slots:
                    nm, pm, pnb = proj_slots[(p, k)]
                    proj_qk_m(nm, kt_sb if nm == "k" else qt,
                              bk_c if nm == "k" else bq_c, pnb, pm)
                if (p, k) in yproj_slots:
                    yproj_i(yproj_slots[(p, k)], yh)
            if p >= 1:
                for kk in range(ST - 3, ST):
                    pv(pairs[p - 1][1], kk, pts_prev[kk], accs_run)
                st = norm_stage1(accs_run,
                                 act_evict=(p == len(pairs) - 1))
                apply_q.append((pairs[p - 1][0], pairs[p - 1][1], st))
            pts_prev = pts_cur

        # ---- tail: PV + norm of the last pair (3,1), split into two
        # 256-col halves so half A's norm chain + yproj(12,13) overlap
        # half B's PV stream.
        qb_l, m_l = pairs[-1]
        accsA = alloc_accs()
        for k in range(ST):
            pv(m_l, k, pts_prev[k], accsA, c0=0, c1=256)
            if k == 5 and apply_q:
                norm_apply(*apply_q.pop(0))
        stA = norm_stage1(accsA, w=256, act_evict=True)

        ssb = ps_s.tile([128, 1024], F32, tag="ss", name="ss")
        accsB = [ssb[0:65, 0:256], ssb[0:65, 512:768]]
        for k in range(ST):
            pv(m_l, k, pts_prev[k], accsB, c0=256, c1=512)
            if k == 3:
                norm_apply(qb_l, m_l, stA, oc=0, w=256)
            if k == 6:
                yproj_i(4 * qb_l + 0, yh, act_evict=True)
            if k == 11:
                yproj_i(4 * qb_l + 1, yh, act_evict=True)
        # endgame: two 128-col pieces of half B so yproj(14) starts
        # while piece 2's norm chain is still in flight
        stB1 = norm_stage1(accsB, w=128, c0=0, act_evict=True)
        norm_apply(qb_l, m_l, stB1, oc=256, w=128)
        stB2 = norm_stage1(accsB, w=128, c0=128, act_evict=True)
        yproj_i(4 * qb_l + 2, yh, act_evict=True)
        norm_apply(qb_l, m_l, stB2, oc=384, w=128)
        yproj_i(4 * qb_l + 3, yh, act_evict=True)


def _get_nc():
    global _cached_nc
    if _cached_nc is None:
        _cached_nc = build_nc()
    return _cached_nc


def _make_in_maps(query, key, value, Wq, bq, Wk, bk, Wv, bv, Wo):
    """Shard + transpose + bf16-cast on host: core c = (b, hg), b = c // HG."""
    query = np.asarray(query, dtype=np.float32)
    key = np.asarray(key, dtype=np.float32)
    value = np.asarray(value, dtype=np.float32)
    Wq, Wk, Wv, Wo = (np.asarray(w, dtype=np.float32) for w in (Wq, Wk, Wv, Wo))
    bq, bk, bv = (np.asarray(b_, dtype=np.float32) for b_ in (bq, bk, bv))
    in_maps = []

    def tile_x(xt, dt):      # [1024, 2048] -> [128, 8*2048] k-tiled
        return np.ascontiguousarray(
            xt.reshape(KT, 128, S).transpose(1, 0, 2).reshape(128, KT * S)
        ).astype(dt)

    xq_t = [tile_x(query[b].T, BF16_NP) for b in range(B)]
    xk_t = [tile_x(key[b].T, BF16_NP) for b in range(B)]
    xv_t = [tile_x(value[b].T, BF16_NP) for b in range(B)]

    def tile_w(WT, dt=BF16_NP):  # [1024, 256] -> [128, 8*256] k-tiled
        return np.ascontiguousarray(
            WT.reshape(KT, 128, DH).transpose(1, 0, 2).reshape(128, KT * DH)
        ).astype(dt)

    for c in range(N_CORES):
        b, hg = divmod(c, HG)
        hs = slice(hg * DH, (hg + 1) * DH)
        wo_tiled = np.ascontiguousarray(
            Wo[:, hs].T.reshape(2, 128, D_MODEL).transpose(1, 0, 2)
            .reshape(128, 2 * D_MODEL)).astype(BF16_NP)
        bqk_pack = np.concatenate(
            [bq[hs].reshape(2, 128).T, bk[hs].reshape(2, 128).T],
            axis=1)          # [128, 4] = bq cols | bk cols
        in_maps.append({
            "xq_t": xq_t[b],
            "xk_t": xk_t[b],
            "xv_t": xv_t[b],
            "wq_t": tile_w(Wq[hs, :].T),
            "wk_t": tile_w(Wk[hs, :].T),
            "wv_t": tile_w(Wv[hs, :].T),
            "wo_t": wo_tiled,
            "bqk": np.ascontiguousarray(bqk_pack),
            "bv": np.ascontiguousarray(bv[hs]).reshape(1, DH).astype(BF16_NP),
        })
    return in_maps


def run(inputs, trace=False, **spmd_kwargs):
    nc = _get_nc()
    in_maps = _make_in_maps(
        inputs["query"], inputs["key"], inputs["value"],
        inputs["Wq"], inputs["bq"], inputs["Wk"], inputs["bk"],
        inputs["Wv"], inputs["bv"], inputs["Wo"])
    res = run_bass_kernel_spmd(
        nc, in_maps, list(range(N_CORES)), trace=trace, **spmd_kwargs)
    bo = np.asarray(inputs["bo"], dtype=np.float32)
    out = np.empty((B, S, D_MODEL), dtype=np.float32)
    for b in range(B):
        acc = np.zeros((S, D_MODEL), dtype=np.float32)
        for hg in range(HG):
            acc += np.asarray(res.results[b * HG + hg]["y"], dtype=np.float32)
        out[b] = acc + bo
    return out, res


def kernel(**inputs) -> np.ndarray:
    out, _ = run(inputs, trace=False)
    return out
